# revision 28
# baseline (speedup 1.0000x reference)
"""Trainium2 Bass kernel: single-layer GRU (T=512, B=64, F=128, H=512) + output proj (O=16).

Sharding: data-parallel over batch. B=64 -> 8 cores x 8 sequences each.
Weights replicated; the recurrence is fully local per core.

Per-core layout (everything "hidden-dim on partitions"):
  x_sb    [128(f), T*8(t,b)]            bf16
  w_ih_sb [128(f), 12*128(g')]          bf16   (gate-chunk-permuted columns)
  w_hh_sb [128(k), 4(hc), 12*128(g')]   bf16
  xg      8 tiles [128(g'p), 12(g'c), 64*8(t,b)] bf16  (precomputed x-side gates + biases)
  hs_sb   [128(hp), T, 4(hc), 8(b)]     bf16   (hidden history, feeds next step's matmul
                                               rhs and the final output projection)

Device gate-chunk order g' = [r0,r1,z0,z1, r2,r3,z2,z3, n0,n1,n2,n3] so that each
"half" of the hidden state (chunks 0-1 / 2-3) has its r/z/n slices contiguous; the
elementwise GRU update runs per-half, letting h(t) half 0 be ready while the PE is
still accumulating half 1 -- the PE never waits on the full elementwise chain.

Recurrence matmul: out[128(g'), 8(b)] += w_hh_sb[:,hc,g'*128:...].T @ h[hc]; the
weight tiles are the stationary operand (bf16 -> fast-weight-load), h the moving one.
"""

import os
import numpy as np
import ml_dtypes
from contextlib import ExitStack

import concourse.bass as bass
import concourse.tile as tile
from concourse import bacc, mybir
from concourse.bass import ds, ts
from concourse.bass_utils import run_bass_kernel_spmd

T, B, F, H, O = 512, 64, 128, 512, 16
N_CORES = 8
BL = B // N_CORES          # 8 sequences per core
GC = (3 * H) // 128        # 12 gate chunks
HC = H // 128              # 4 hidden chunks
TCH = 8                    # xg is staged in 8 chunks of 64 timesteps
TC = T // TCH              # 64
# device gate-chunk order = original order [r0..r3, z0..z3, n0..n3]
PERM_BLOCKS = list(range(12))

F32 = mybir.dt.float32
BF16 = mybir.dt.bfloat16
BF_NP = ml_dtypes.bfloat16


def build_nc(t_steps: int = T, reps: int = 1):
    """Build + compile the per-core Bass program (SPMD: same program, 8 cores).

    reps > 1 wraps the whole body in a device-side For_i loop (identical
    iterations) so per-execution time can be measured without per-dispatch
    host/tunnel overhead."""
    from contextlib import nullcontext
    FT = mybir.ActivationFunctionType
    nc = bacc.Bacc("TRN2", target_bir_lowering=False, debug=False,
                   num_devices=N_CORES)

    x_in = nc.dram_tensor("x", [128, T * BL], BF16, kind="ExternalInput")
    whh_in = nc.dram_tensor("w_hh_t", [HC, 128, GC * 128], BF16, kind="ExternalInput")
    wih_in = nc.dram_tensor("w_ih_t", [128, GC * 128], BF16, kind="ExternalInput")
    bias_in = nc.dram_tensor("biasg", [128, GC], F32, kind="ExternalInput")
    bhn_in = nc.dram_tensor("bhn_bc", [128, HC * BL], BF16, kind="ExternalInput")
    ident_in = nc.dram_tensor("ident", [128, 128], BF16, kind="ExternalInput")
    wout_in = nc.dram_tensor("w_out_t", [HC, 128, O], BF16, kind="ExternalInput")
    bout_in = nc.dram_tensor("b_out_p", [O, 1], F32, kind="ExternalInput")
    y_out = nc.dram_tensor("y", [O, T * BL], F32, kind="ExternalOutput")

    with tile.TileContext(nc) as tc, ExitStack() as ctx:
      with (tc.For_i(0, reps, 1) if reps > 1 else nullcontext()):
        const = ctx.enter_context(tc.tile_pool(name="const", bufs=1))
        psum = ctx.enter_context(tc.tile_pool(name="psum", bufs=2, space="PSUM"))
        work = ctx.enter_context(tc.tile_pool(name="work", bufs=2))

        # ---- constants / inputs to SBUF
        x_sb = const.tile([128, T * BL], BF16)
        nc.sync.dma_start(x_sb[:], x_in.ap()[:])
        whh_sb = const.tile([128, HC, GC * 128], BF16)
        for hc in range(HC):
            nc.sync.dma_start(whh_sb[:, hc, :], whh_in.ap()[hc])
        wih_sb = const.tile([128, GC * 128], BF16)
        nc.sync.dma_start(wih_sb[:], wih_in.ap()[:])
        bias_sb = const.tile([128, GC], F32)
        nc.sync.dma_start(bias_sb[:], bias_in.ap()[:])
        bhn_sb = const.tile([128, HC, BL], BF16)
        nc.sync.dma_start(bhn_sb[:], bhn_in.ap()[:])
        ident_sb = const.tile([128, 128], BF16)
        nc.sync.dma_start(ident_sb[:], ident_in.ap()[:])
        wout_sb = const.tile([128, HC, O], BF16)
        for hc in range(HC):
            nc.sync.dma_start(wout_sb[:, hc, :], wout_in.ap()[hc])
        bout_sb = const.tile([O, 1], F32)
        nc.sync.dma_start(bout_sb[:], bout_in.ap()[:])

        hs_sb = const.tile([128, T, HC, BL], BF16)
        h0_bf = const.tile([128, HC, BL], BF16)
        nc.vector.memset(h0_bf[:], 0)
        xg_tiles = [const.tile([128, GC, TC * BL], BF16, name=f"xg{i}")
                    for i in range(TCH)]

        # ---- phase 1: xg[g', (t,b)] = w_ih' . x + (b_ih + b_hh)  (permuted gate order)
        for c in range(TCH):
            for g in range(GC):
                ps = psum.tile([128, TC * BL], F32, tag=f"p{g % 4}")
                nc.tensor.matmul(ps[:], wih_sb[:, ts(g, 128)],
                                 x_sb[:, ts(c, TC * BL)], start=True, stop=True)
                dst = xg_tiles[c][:, g, :]
                if g % 2 == 0:
                    nc.scalar.activation(dst, ps[:], FT.Identity,
                                         bias=bias_sb[:, g:g + 1], scale=1.0)
                else:
                    nc.vector.tensor_scalar_add(dst, ps[:], bias_sb[:, g:g + 1])

        # ---- phase 2: the recurrence
        # Per half h: rz gates g' 4h..4h+3 -> ps_rz[h], n gates 8+2h..9+2h ->
        # ps_n[h] (b_hh^n folded in via a K=1 matmul against a ones row).
        # Chain per half: rz-add (DVE, in-place on PSUM) -> sigmoid (ACT,
        # PSUM src) -> n-mult/add (DVE) -> tanh (ACT) -> d/e/h (GpSimd) with
        # h written straight to hs_sb as bf16 (no f32 h, no extra copy).
        for t in range(t_steps):
            c, tt = divmod(t, TC)
            xg = xg_tiles[c]
            if t == 0:
                rhs = h0_bf
            else:
                rhs = hs_sb[:, t - 1, :, :]

            ps_rz = psum.tile([128, 8, BL], F32, tag="p0", name="ps_rz")
            ps_n = psum.tile([128, 4, BL], F32, tag="p1", name="ps_n")

            tb = ds(tt * BL, BL)
            # start=True clears the whole PSUM bank, so emit exactly one
            # accumulation group per psum tile (start only on the tile's
            # first matmul of the step).
            # xg (rz gates) and b_hh^n are injected into PSUM through the
            # PE with an identity stationary (psum += I.T @ v = v): these
            # matmuls depend only on phase-1 data, so they fill the PE's
            # idle window while it waits for h(t-1), and they remove the
            # DVE pre-activation add (and its two chain hops) entirely.
            for j in range(8):
                nc.tensor.matmul(ps_rz[:, j, :], ident_sb[:],
                                 xg[:, j, tb],
                                 start=(j == 0), stop=False,
                                 skip_group_check=True)
            for j in range(4):
                nc.tensor.matmul(ps_n[:, j, :], ident_sb[:],
                                 bhn_sb[:, j, :],
                                 start=(j == 0), stop=False,
                                 skip_group_check=True)
            # all rz matmuls first so ps_rz stops (and the chain starts) as
            # early as possible; n matmuls follow. hc-pair outer: the (0,1)
            # contraction block only needs the first half of h(t-1), which the
            # split h'-write below makes available one DVE op earlier.
            for hcpair in ((0, 1), (2, 3)):
                for j in range(8):
                    for hc in hcpair:
                        nc.tensor.matmul(ps_rz[:, j, :],
                                         whh_sb[:, hc, ts(j, 128)],
                                         rhs[:, hc, :],
                                         start=False,
                                         stop=(hc == 3 and j == 7),
                                         skip_group_check=True)
            for hcpair in ((0, 1), (2, 3)):
                for j in range(4):
                    for hc in hcpair:
                        nc.tensor.matmul(ps_n[:, j, :],
                                         whh_sb[:, hc, ts(8 + j, 128)],
                                         rhs[:, hc, :],
                                         start=False,
                                         stop=(hc == 3 and j == 3),
                                         skip_group_check=True)

            rs = work.tile([128, 8, BL], BF16, tag="rs")
            # One full-tile sigmoid: chunks 0:4 are r, chunks 4:8 hold
            # z' = 1 - z (z-gate weights/biases negated host-side).
            # Must read the WHOLE psum tile: a subtile read would race the
            # remaining same-bank matmul writes (fatal PSUM collision).
            nc.scalar.activation(rs[:], ps_rz[:], FT.Sigmoid)
            nm = work.tile([128, 4, BL], BF16, tag="nm")
            nc.vector.tensor_mul(nm[:], ps_n[:], rs[:, 0:4, :])
            np_ = work.tile([128, 4, BL], BF16, tag="np")
            nc.vector.tensor_add(np_[:], nm[:], xg[:, 8:12, tb])
            nt = work.tile([128, 4, BL], BF16, tag="nt")
            nc.scalar.activation(nt[:], np_[:], FT.Tanh)
            # h' = z'*n + u,  u = h - z'*h  (= (1-z)n + z*h).  u needs only
            # z' and h(t-1), so it runs on idle GpSimd in parallel with the
            # tanh path; just 2 serial ops remain after tanh.
            u1 = work.tile([128, HC, BL], BF16, tag="u1")
            nc.gpsimd.tensor_mul(u1[:], rhs[:, :, :], rs[:, 4:8, :])
            u = work.tile([128, HC, BL], BF16, tag="u")
            nc.gpsimd.tensor_sub(u[:], rhs[:, :, :], u1[:])
            # tail split by hidden-chunk half: h'[0:2] lands first so the next
            # step's hc(0,1) matmuls can begin while h'[2:4] still computes.
            q = work.tile([128, HC, BL], BF16, tag="q")
            nc.vector.tensor_mul(q[:, 0:2, :], nt[:, 0:2, :], rs[:, 4:6, :])
            nc.vector.tensor_add(hs_sb[:, t, 0:2, :], q[:, 0:2, :],
                                 u[:, 0:2, :])
            nc.vector.tensor_mul(q[:, 2:4, :], nt[:, 2:4, :], rs[:, 6:8, :])
            nc.vector.tensor_add(hs_sb[:, t, 2:4, :], q[:, 2:4, :],
                                 u[:, 2:4, :])

        # ---- phase 3: y = w_out . h_t + b_out
        for c in range(TCH):
            ps = psum.tile([O, TC * BL], F32, tag="p0")
            for hc in range(HC):
                nc.tensor.matmul(ps[:], wout_sb[:, hc, :],
                                 hs_sb[:, ts(c, TC), hc, :],
                                 start=(hc == 0), stop=(hc == 3))
            yt = work.tile([O, TC * BL], F32, tag="yt")
            nc.scalar.activation(yt[:], ps[:], FT.Identity, bias=bout_sb[:],
                                 scale=1.0)
            nc.sync.dma_start(y_out.ap()[:, ts(c, TC * BL)], yt[:])

    nc.compile()
    return nc


def prep_inputs(x_rnn, w_ih, w_hh, b_ih, b_hh, w_out, b_out):
    """Host-side shard + relayout. Returns per-core in_maps."""
    x_rnn = np.asarray(x_rnn, np.float32)
    w_ih = np.asarray(w_ih, np.float32)
    w_hh = np.asarray(w_hh, np.float32)
    b_ih = np.asarray(b_ih, np.float32)
    b_hh = np.asarray(b_hh, np.float32)
    w_out = np.asarray(w_out, np.float32)
    b_out = np.asarray(b_out, np.float32)

    rows = np.concatenate([np.arange(b * 128, (b + 1) * 128) for b in PERM_BLOCKS])
    w_ih_p = w_ih[rows].copy()                # (1536, 128), permuted gate order
    w_hh_p = w_hh[rows].copy()                # (1536, 512)
    # r/z gates: fold both biases into xg. n gates: only b_ih (b_hn lives
    # inside the r* product and is applied during the recurrence).
    bsum = (b_ih + b_hh)[rows]
    bsum[8 * 128:] = b_ih[rows][8 * 128:]
    # negate the z-gate so sigmoid yields z' = 1 - z on device (see kernel).
    # device chunk order: [r0..r3, z0..z3, n0..n3]; z chunks at 4..7
    for zc in (4, 5, 6, 7):
        w_ih_p[zc * 128:(zc + 1) * 128] *= -1.0
        w_hh_p[zc * 128:(zc + 1) * 128] *= -1.0
        bsum[zc * 128:(zc + 1) * 128] *= -1.0
    biasg = bsum.reshape(GC, 128).T.copy()                      # (128, GC) f32
    # b_hh^n broadcast over batch: [128 g_row, HC, BL]
    bhn_bc = np.repeat(b_hh[2 * H:].reshape(HC, 128).T[:, :, None], BL, axis=2)
    ident = np.eye(128, dtype=np.float32)

    w_ih_t = np.ascontiguousarray(w_ih_p.T).astype(BF_NP)       # (128, 1536)
    w_hh_t = np.ascontiguousarray(w_hh_p.T.reshape(HC, 128, GC * 128)).astype(BF_NP)
    w_out_t = np.ascontiguousarray(w_out.T.reshape(HC, 128, O)).astype(BF_NP)
    b_out_p = b_out.reshape(O, 1).astype(np.float32)

    in_maps = []
    for c in range(N_CORES):
        xc = x_rnn[:, c * BL:(c + 1) * BL, :]             # (T, 8, 128)
        x_t = np.ascontiguousarray(xc.transpose(2, 0, 1).reshape(128, T * BL))
        in_maps.append({
            "x": x_t.astype(BF_NP),
            "w_hh_t": w_hh_t, "w_ih_t": w_ih_t, "biasg": biasg.astype(np.float32),
            "bhn_bc": bhn_bc.reshape(128, HC * BL).astype(BF_NP),
            "ident": ident.astype(BF_NP),
            "w_out_t": w_out_t, "b_out_p": b_out_p,
        })
    return in_maps


def assemble_output(results):
    """results: list of per-core {"y": (O, T*BL)} -> full (T, B, O) f32."""
    ys = []
    for c in range(N_CORES):
        yc = np.asarray(results[c]["y"], np.float32)
        ys.append(yc.reshape(O, T, BL).transpose(1, 2, 0))
    return np.concatenate(ys, axis=1)


_NC_CACHE = {}


def get_nc(t_steps: int = T, reps: int = 1):
    if (t_steps, reps) not in _NC_CACHE:
        _NC_CACHE[(t_steps, reps)] = build_nc(t_steps, reps)
    return _NC_CACHE[(t_steps, reps)]


def kernel(**inputs) -> np.ndarray:
    nc = get_nc()
    in_maps = prep_inputs(**inputs)
    res = run_bass_kernel_spmd(nc, in_maps, list(range(N_CORES)))
    return assemble_output(res.results)



# revision 29
# speedup vs baseline: 1.0020x; 1.0020x over previous
"""Trainium2 Bass kernel: single-layer GRU (T=512, B=64, F=128, H=512) + output proj (O=16).

Sharding: data-parallel over batch. B=64 -> 8 cores x 8 sequences each.
Weights replicated; the recurrence is fully local per core.

Per-core layout (everything "hidden-dim on partitions"):
  x_sb    [128(f), T*8(t,b)]            bf16
  w_ih_sb [128(f), 12*128(g')]          bf16   (gate-chunk-permuted columns)
  w_hh_sb [128(k), 4(hc), 12*128(g')]   bf16
  xg      8 tiles [128(g'p), 12(g'c), 64*8(t,b)] bf16  (precomputed x-side gates + biases)
  hs_sb   [128(hp), T, 4(hc), 8(b)]     bf16   (hidden history, feeds next step's matmul
                                               rhs and the final output projection)

Device gate-chunk order g' = [r0,r1,z0,z1, r2,r3,z2,z3, n0,n1,n2,n3] so that each
"half" of the hidden state (chunks 0-1 / 2-3) has its r/z/n slices contiguous; the
elementwise GRU update runs per-half, letting h(t) half 0 be ready while the PE is
still accumulating half 1 -- the PE never waits on the full elementwise chain.

Recurrence matmul: out[128(g'), 8(b)] += w_hh_sb[:,hc,g'*128:...].T @ h[hc]; the
weight tiles are the stationary operand (bf16 -> fast-weight-load), h the moving one.
"""

import os
import numpy as np
import ml_dtypes
from contextlib import ExitStack

import concourse.bass as bass
import concourse.tile as tile
from concourse import bacc, mybir
from concourse.bass import ds, ts
from concourse.bass_utils import run_bass_kernel_spmd

T, B, F, H, O = 512, 64, 128, 512, 16
N_CORES = 8
BL = B // N_CORES          # 8 sequences per core
GC = (3 * H) // 128        # 12 gate chunks
HC = H // 128              # 4 hidden chunks
TCH = 8                    # xg is staged in 8 chunks of 64 timesteps
TC = T // TCH              # 64
# device gate-chunk order = original order [r0..r3, z0..z3, n0..n3]
PERM_BLOCKS = list(range(12))

F32 = mybir.dt.float32
BF16 = mybir.dt.bfloat16
BF_NP = ml_dtypes.bfloat16


def build_nc(t_steps: int = T, reps: int = 1):
    """Build + compile the per-core Bass program (SPMD: same program, 8 cores).

    reps > 1 wraps the whole body in a device-side For_i loop (identical
    iterations) so per-execution time can be measured without per-dispatch
    host/tunnel overhead."""
    from contextlib import nullcontext
    FT = mybir.ActivationFunctionType
    nc = bacc.Bacc("TRN2", target_bir_lowering=False, debug=False,
                   num_devices=N_CORES)

    x_in = nc.dram_tensor("x", [128, T * BL], BF16, kind="ExternalInput")
    whh_in = nc.dram_tensor("w_hh_t", [HC, 128, GC * 128], BF16, kind="ExternalInput")
    wih_in = nc.dram_tensor("w_ih_t", [128, GC * 128], BF16, kind="ExternalInput")
    bias_in = nc.dram_tensor("biasg", [128, GC], F32, kind="ExternalInput")
    bhn_in = nc.dram_tensor("bhn_bc", [128, HC * BL], BF16, kind="ExternalInput")
    ident_in = nc.dram_tensor("ident", [128, 128], BF16, kind="ExternalInput")
    wout_in = nc.dram_tensor("w_out_t", [HC, 128, O], BF16, kind="ExternalInput")
    bout_in = nc.dram_tensor("b_out_p", [O, 1], F32, kind="ExternalInput")
    y_out = nc.dram_tensor("y", [O, T * BL], F32, kind="ExternalOutput")

    with tile.TileContext(nc) as tc, ExitStack() as ctx:
      with (tc.For_i(0, reps, 1) if reps > 1 else nullcontext()):
        const = ctx.enter_context(tc.tile_pool(name="const", bufs=1))
        psum = ctx.enter_context(tc.tile_pool(name="psum", bufs=2, space="PSUM"))
        work = ctx.enter_context(tc.tile_pool(name="work", bufs=2))

        # ---- constants / inputs to SBUF
        x_sb = const.tile([128, T * BL], BF16)
        nc.sync.dma_start(x_sb[:], x_in.ap()[:])
        whh_sb = const.tile([128, HC, GC * 128], BF16)
        for hc in range(HC):
            nc.sync.dma_start(whh_sb[:, hc, :], whh_in.ap()[hc])
        wih_sb = const.tile([128, GC * 128], BF16)
        nc.sync.dma_start(wih_sb[:], wih_in.ap()[:])
        bias_sb = const.tile([128, GC], F32)
        nc.sync.dma_start(bias_sb[:], bias_in.ap()[:])
        bhn_sb = const.tile([128, HC, BL], BF16)
        nc.sync.dma_start(bhn_sb[:], bhn_in.ap()[:])
        ident_sb = const.tile([128, 128], BF16)
        nc.sync.dma_start(ident_sb[:], ident_in.ap()[:])
        wout_sb = const.tile([128, HC, O], BF16)
        for hc in range(HC):
            nc.sync.dma_start(wout_sb[:, hc, :], wout_in.ap()[hc])
        bout_sb = const.tile([O, 1], F32)
        nc.sync.dma_start(bout_sb[:], bout_in.ap()[:])

        hs_sb = const.tile([128, T, HC, BL], BF16)
        h0_bf = const.tile([128, HC, BL], BF16)
        nc.vector.memset(h0_bf[:], 0)
        xg_tiles = [const.tile([128, GC, TC * BL], BF16, name=f"xg{i}")
                    for i in range(TCH)]

        # ---- phase 1: xg[g', (t,b)] = w_ih' . x + (b_ih + b_hh)  (permuted gate order)
        for c in range(TCH):
            for g in range(GC):
                ps = psum.tile([128, TC * BL], F32, tag=f"p{g % 4}")
                nc.tensor.matmul(ps[:], wih_sb[:, ts(g, 128)],
                                 x_sb[:, ts(c, TC * BL)], start=True, stop=True)
                dst = xg_tiles[c][:, g, :]
                if g % 2 == 0:
                    nc.scalar.activation(dst, ps[:], FT.Identity,
                                         bias=bias_sb[:, g:g + 1], scale=1.0)
                else:
                    nc.vector.tensor_scalar_add(dst, ps[:], bias_sb[:, g:g + 1])

        # ---- phase 2: the recurrence
        # Per half h: rz gates g' 4h..4h+3 -> ps_rz[h], n gates 8+2h..9+2h ->
        # ps_n[h] (b_hh^n folded in via a K=1 matmul against a ones row).
        # Chain per half: rz-add (DVE, in-place on PSUM) -> sigmoid (ACT,
        # PSUM src) -> n-mult/add (DVE) -> tanh (ACT) -> d/e/h (GpSimd) with
        # h written straight to hs_sb as bf16 (no f32 h, no extra copy).
        for t in range(t_steps):
            c, tt = divmod(t, TC)
            xg = xg_tiles[c]
            if t == 0:
                rhs = h0_bf
            else:
                rhs = hs_sb[:, t - 1, :, :]

            ps_rz = psum.tile([128, 8, BL], F32, tag="p0", name="ps_rz")
            ps_n = psum.tile([128, 4, BL], F32, tag="p1", name="ps_n")

            tb = ds(tt * BL, BL)
            # start=True clears the whole PSUM bank, so emit exactly one
            # accumulation group per psum tile (start only on the tile's
            # first matmul of the step).
            # xg (rz gates) and b_hh^n are injected into PSUM through the
            # PE with an identity stationary (psum += I.T @ v = v): these
            # matmuls depend only on phase-1 data, so they fill the PE's
            # idle window while it waits for h(t-1), and they remove the
            # DVE pre-activation add (and its two chain hops) entirely.
            for j in range(8):
                nc.tensor.matmul(ps_rz[:, j, :], ident_sb[:],
                                 xg[:, j, tb],
                                 start=(j == 0), stop=False,
                                 skip_group_check=True)
            for j in range(4):
                nc.tensor.matmul(ps_n[:, j, :], ident_sb[:],
                                 bhn_sb[:, j, :],
                                 start=(j == 0), stop=False,
                                 skip_group_check=True)
            # all rz matmuls first so ps_rz stops (and the chain starts) as
            # early as possible; n matmuls follow. hc-pair outer: the (0,1)
            # contraction block only needs the first half of h(t-1), which the
            # split h'-write below makes available one DVE op earlier.
            for hcpair in ((0, 1), (2, 3)):
                for j in range(8):
                    for hc in hcpair:
                        nc.tensor.matmul(ps_rz[:, j, :],
                                         whh_sb[:, hc, ts(j, 128)],
                                         rhs[:, hc, :],
                                         start=False,
                                         stop=(hc == 3 and j == 7),
                                         skip_group_check=True)
            for hcpair in ((0, 1), (2, 3)):
                for j in range(4):
                    for hc in hcpair:
                        nc.tensor.matmul(ps_n[:, j, :],
                                         whh_sb[:, hc, ts(8 + j, 128)],
                                         rhs[:, hc, :],
                                         start=False,
                                         stop=(hc == 3 and j == 3),
                                         skip_group_check=True)

            rs = work.tile([128, 8, BL], BF16, tag="rs")
            # One full-tile sigmoid: chunks 0:4 are r, chunks 4:8 hold
            # z' = 1 - z (z-gate weights/biases negated host-side).
            # Must read the WHOLE psum tile: a subtile read would race the
            # remaining same-bank matmul writes (fatal PSUM collision).
            nc.scalar.activation(rs[:], ps_rz[:], FT.Sigmoid)
            nm = work.tile([128, 4, BL], BF16, tag="nm")
            nc.vector.tensor_mul(nm[:], ps_n[:], rs[:, 0:4, :])
            np_ = work.tile([128, 4, BL], BF16, tag="np")
            nc.vector.tensor_add(np_[:], nm[:], xg[:, 8:12, tb])
            nt = work.tile([128, 4, BL], BF16, tag="nt")
            nc.scalar.activation(nt[:], np_[:], FT.Tanh)
            # h' = z'*n + u,  u = h - z'*h  (= (1-z)n + z*h).  u needs only
            # z' and h(t-1), so it runs on idle GpSimd in parallel with the
            # tanh path; just 2 serial ops remain after tanh.
            u1 = work.tile([128, HC, BL], BF16, tag="u1")
            nc.gpsimd.tensor_mul(u1[:], rhs[:, :, :], rs[:, 4:8, :])
            u = work.tile([128, HC, BL], BF16, tag="u")
            nc.gpsimd.tensor_sub(u[:], rhs[:, :, :], u1[:])
            q = work.tile([128, HC, BL], BF16, tag="q")
            nc.vector.tensor_mul(q[:], nt[:], rs[:, 4:8, :])
            nc.vector.tensor_add(hs_sb[:, t, :, :], q[:], u[:])

        # ---- phase 3: y = w_out . h_t + b_out
        for c in range(TCH):
            ps = psum.tile([O, TC * BL], F32, tag="p0")
            for hc in range(HC):
                nc.tensor.matmul(ps[:], wout_sb[:, hc, :],
                                 hs_sb[:, ts(c, TC), hc, :],
                                 start=(hc == 0), stop=(hc == 3))
            yt = work.tile([O, TC * BL], F32, tag="yt")
            nc.scalar.activation(yt[:], ps[:], FT.Identity, bias=bout_sb[:],
                                 scale=1.0)
            nc.sync.dma_start(y_out.ap()[:, ts(c, TC * BL)], yt[:])

    nc.compile()
    return nc


def prep_inputs(x_rnn, w_ih, w_hh, b_ih, b_hh, w_out, b_out):
    """Host-side shard + relayout. Returns per-core in_maps."""
    x_rnn = np.asarray(x_rnn, np.float32)
    w_ih = np.asarray(w_ih, np.float32)
    w_hh = np.asarray(w_hh, np.float32)
    b_ih = np.asarray(b_ih, np.float32)
    b_hh = np.asarray(b_hh, np.float32)
    w_out = np.asarray(w_out, np.float32)
    b_out = np.asarray(b_out, np.float32)

    rows = np.concatenate([np.arange(b * 128, (b + 1) * 128) for b in PERM_BLOCKS])
    w_ih_p = w_ih[rows].copy()                # (1536, 128), permuted gate order
    w_hh_p = w_hh[rows].copy()                # (1536, 512)
    # r/z gates: fold both biases into xg. n gates: only b_ih (b_hn lives
    # inside the r* product and is applied during the recurrence).
    bsum = (b_ih + b_hh)[rows]
    bsum[8 * 128:] = b_ih[rows][8 * 128:]
    # negate the z-gate so sigmoid yields z' = 1 - z on device (see kernel).
    # device chunk order: [r0..r3, z0..z3, n0..n3]; z chunks at 4..7
    for zc in (4, 5, 6, 7):
        w_ih_p[zc * 128:(zc + 1) * 128] *= -1.0
        w_hh_p[zc * 128:(zc + 1) * 128] *= -1.0
        bsum[zc * 128:(zc + 1) * 128] *= -1.0
    biasg = bsum.reshape(GC, 128).T.copy()                      # (128, GC) f32
    # b_hh^n broadcast over batch: [128 g_row, HC, BL]
    bhn_bc = np.repeat(b_hh[2 * H:].reshape(HC, 128).T[:, :, None], BL, axis=2)
    ident = np.eye(128, dtype=np.float32)

    w_ih_t = np.ascontiguousarray(w_ih_p.T).astype(BF_NP)       # (128, 1536)
    w_hh_t = np.ascontiguousarray(w_hh_p.T.reshape(HC, 128, GC * 128)).astype(BF_NP)
    w_out_t = np.ascontiguousarray(w_out.T.reshape(HC, 128, O)).astype(BF_NP)
    b_out_p = b_out.reshape(O, 1).astype(np.float32)

    in_maps = []
    for c in range(N_CORES):
        xc = x_rnn[:, c * BL:(c + 1) * BL, :]             # (T, 8, 128)
        x_t = np.ascontiguousarray(xc.transpose(2, 0, 1).reshape(128, T * BL))
        in_maps.append({
            "x": x_t.astype(BF_NP),
            "w_hh_t": w_hh_t, "w_ih_t": w_ih_t, "biasg": biasg.astype(np.float32),
            "bhn_bc": bhn_bc.reshape(128, HC * BL).astype(BF_NP),
            "ident": ident.astype(BF_NP),
            "w_out_t": w_out_t, "b_out_p": b_out_p,
        })
    return in_maps


def assemble_output(results):
    """results: list of per-core {"y": (O, T*BL)} -> full (T, B, O) f32."""
    ys = []
    for c in range(N_CORES):
        yc = np.asarray(results[c]["y"], np.float32)
        ys.append(yc.reshape(O, T, BL).transpose(1, 2, 0))
    return np.concatenate(ys, axis=1)


_NC_CACHE = {}


def get_nc(t_steps: int = T, reps: int = 1):
    if (t_steps, reps) not in _NC_CACHE:
        _NC_CACHE[(t_steps, reps)] = build_nc(t_steps, reps)
    return _NC_CACHE[(t_steps, reps)]


def kernel(**inputs) -> np.ndarray:
    nc = get_nc()
    in_maps = prep_inputs(**inputs)
    res = run_bass_kernel_spmd(nc, in_maps, list(range(N_CORES)))
    return assemble_output(res.results)

